# revision 82
# baseline (speedup 1.0000x reference)
"""Multi-head attention (RoPE, causal) Trainium2 Bass kernel, 8-way sharded.

Sharding: core c handles batch b = c//4 and heads 4*(c%4)..4*(c%4)+3
(B*H = 32 head-rows -> 4 per core).  QKV/out projections are
Megatron-sliced per core; per-core partial outputs (row-parallel Wo)
are summed on the host.

v3 (all-bf16; fp8 tested ~4%/GEMM output error — over the 2e-2 budget):
 - AV emits [q, 65] accumulators (full PE width + per-partition softmax
   denominators) instead of [65, q]: halves AV PE time and turns
   normalization into reciprocal + tensor_scalar (no DRAM bounce).
 - Normalized attention is DMA-transposed (XBAR) into Wo-ready layout.
 - Next chunk's projection/rope is woven into attention pair boundaries;
   AV is software-pipelined one step behind exp so the in-order PE
   doesn't stall on the Activation engine.
 - Causal trims: per-dj leading-column skip on scores, exp split around
   masked spans, 128-col staircase mask multiply (vs 512-col).

Problem constants (hardcoded per contract):
  B=2, S=2048, D=1024, H=16, DK=64
"""

import math

import ml_dtypes
import numpy as np

import concourse.bass as bass
import concourse.mybir as mybir
import concourse.tile as tile
from concourse import bacc
from concourse.bass_utils import run_bass_kernel_spmd

B, S, D, H, DK = 2, 2048, 1024, 16, 64
E = 256            # head dims per core (4 heads x 64)
CH = 512           # sequence chunk (matmul free dim)
NCH = S // CH      # 4
NST = S // 128     # 16 s-tiles
BF16 = mybir.dt.bfloat16
F32 = mybir.dt.float32
# schedule tuning (selected via TimelineSim sweeps)
EARLY_NORM = False   # per-qt normalize emission loses to batched-at-boundary
PROJ_SHIFT = False   # projection unit half-boundary earlier regressed
AV_DEPTH = 2         # AV software-pipeline depth behind exp


def _np_reference_fallback(q, k, v, mask, Wq, bq, Wk, bk, Wv, bv, Wo, bo):
    """Pure-numpy reference path (only used for inputs outside the
    contract: non-causal mask or nonzero qkv biases)."""
    qh = (q @ Wq.T + bq).reshape(B, S, H, DK)
    kh = (k @ Wk.T + bk).reshape(B, S, H, DK)
    vh = (v @ Wv.T + bv).reshape(B, S, H, DK)
    inv_freq = 1.0 / (10000.0 ** (np.arange(0, DK, 2, dtype=np.float32) / DK))
    pos = np.arange(S, dtype=np.float32)
    fr = pos[:, None] * inv_freq[None, :]
    cos, sin = np.cos(fr)[:, None, :], np.sin(fr)[:, None, :]

    def rope(x):
        t = DK // 2
        x1, x2 = x[..., :t], x[..., t:]
        return np.concatenate([x1 * cos - x2 * sin, x1 * sin + x2 * cos], -1)

    qh, kh = rope(qh), rope(kh)
    sc = np.einsum('bqhd,bkhd->bhqk', qh, kh) / math.sqrt(DK)
    sc = np.where(mask == 0, np.float32(-10000.0), sc)
    sc = sc - sc.max(-1, keepdims=True)
    e = np.exp(sc)
    attn = e / e.sum(-1, keepdims=True)
    out = np.einsum('bhqk,bkhd->bqhd', attn, vh).reshape(B, S, D)
    return (out @ Wo.T + bo).astype(np.float32)


def _build_program():
    nc = bacc.Bacc(None, target_bir_lowering=False)

    dp = nc.declare_dram_parameter
    xq = dp("xq", [D, S], BF16, isOutput=False)   # q[b].T
    xk = dp("xk", [D, S], BF16, isOutput=False)
    xv = dp("xv", [D, S], BF16, isOutput=False)
    wq = dp("wq", [D, E], BF16, isOutput=False)   # Wq_c.T
    wk = dp("wk", [D, E], BF16, isOutput=False)
    wv = dp("wv", [D, E], BF16, isOutput=False)
    wo = dp("wo", [E, D], BF16, isOutput=False)   # Wo_c.T rows
    ct = dp("ct", [E, S], BF16, isOutput=False)   # cos table (1/sqrt8 folded)
    st = dp("st", [E, S], BF16, isOutput=False)   # signed sin table
    rt = dp("rt", [128, 128], BF16, isOutput=False)  # half-swap permutation
    lt = dp("lt", [128, 128], BF16, isOutput=False)  # lower-tri (p<=w) mask
    out = dp("out", [S, D], BF16, isOutput=True)

    with tile.TileContext(nc) as tc:
        with (
            tc.tile_pool(name="const", bufs=1) as const,
            tc.tile_pool(name="persist", bufs=1) as persist,
            tc.tile_pool(name="xt", bufs=6) as xtp,
            tc.tile_pool(name="raw", bufs=4) as rawp,
            tc.tile_pool(name="ropetmp", bufs=4) as rtp,
            tc.tile_pool(name="pblk", bufs=10) as pblk,
            tc.tile_pool(name="aq", bufs=8) as aqp,
            tc.tile_pool(name="norm", bufs=8) as normp,
            tc.tile_pool(name="obuf", bufs=4) as obufp,
            tc.tile_pool(name="projps", bufs=2, space="PSUM") as projps,
        ):
            # ---- constants to SBUF ----
            rt_t = const.tile([128, 128], BF16, tag="rt")
            nc.gpsimd.dma_start(out=rt_t[:], in_=rt[:])
            wk_t = const.tile([128, 8, E], BF16, tag="wk")
            nc.sync.dma_start(out=wk_t[:], in_=wk[:].rearrange("(kt p) e -> p kt e", p=128))
            wq_t = const.tile([128, 8, E], BF16, tag="wq")
            ct_t = const.tile([128, 2, S], BF16, tag="ct")
            st_t = const.tile([128, 2, S], BF16, tag="st")
            wv_t = const.tile([128, 8, E], BF16, tag="wv")
            lt_t = const.tile([128, 128], BF16, tag="lt")
            wo_t = const.tile([128, 2, D], BF16, tag="wo")

            # ---- persistent intermediates ----
            wt = const.tile([128, 128], BF16, tag="wt")
            nc.vector.memset(wt[:], 1.0)
            qT = persist.tile([128, 2, S], BF16, tag="qT")   # partitions: e%128, dim1: e//128
            kT = persist.tile([128, 2, S], BF16, tag="kT")
            aT = persist.tile([128, 2, S], BF16, tag="aT")
            vext = persist.tile([128, NST, 4, 65], BF16, tag="vext")
            nc.vector.memset(vext[:, :, :, 64:65], 1.0)

            def load_x_chunk(x_dram, c, dma_eng, split=False):
                xt = xtp.tile([128, 8, CH], BF16, tag="xt")
                xsrc = x_dram[:].rearrange("(kt p) s -> p kt s", p=128)
                if split:
                    # two DMAs -> transfers run on separate DMA engines
                    dma_eng.dma_start(out=xt[:, 0:4, :], in_=xsrc[:, 0:4, c * CH:(c + 1) * CH])
                    dma_eng.dma_start(out=xt[:, 4:8, :], in_=xsrc[:, 4:8, c * CH:(c + 1) * CH])
                else:
                    dma_eng.dma_start(out=xt[:], in_=xsrc[:, :, c * CH:(c + 1) * CH])
                return xt

            def proj_mm(xt, w_t, m):
                """Projection matmuls + PSUM->SBUF evict; returns raw tile.
                Split from the rope finish so several m-units' matmuls can be
                emitted back-to-back (the PE never waits on the evict)."""
                ps = projps.tile([128, CH], F32, tag="ps")
                for kt in range(8):
                    nc.tensor.matmul(
                        ps[:], lhsT=w_t[:, kt, m * 128:(m + 1) * 128],
                        rhs=xt[:, kt, :], start=(kt == 0), stop=(kt == 7),
                    )
                raw = rawp.tile([128, CH], BF16, tag="raw")
                nc.vector.tensor_copy(raw[:], ps[:])
                return raw

            def rope_finish(raw, dest, c, m, eng=None):
                eng = eng or nc.gpsimd
                rps = projps.tile([128, CH], F32, tag="ps")
                nc.tensor.matmul(rps[:], lhsT=rt_t[:], rhs=raw[:], start=True, stop=True)
                t1 = rtp.tile([128, CH], BF16, tag="rtmp")
                nc.vector.tensor_mul(t1[:], rps[:], st_t[:, m, c * CH:(c + 1) * CH])
                t2 = rtp.tile([128, CH], BF16, tag="rtmp")
                eng.tensor_mul(t2[:], raw[:], ct_t[:, m, c * CH:(c + 1) * CH])
                eng.tensor_add(dest[:, m, c * CH:(c + 1) * CH], t1[:], t2[:])

            def proj_rope_pair(xt_k, xt_q, c, m, split=True, eng=None):
                if split:
                    raw_k = proj_mm(xt_k, wk_t, m)
                    raw_q = proj_mm(xt_q, wq_t, m)
                    rope_finish(raw_k, kT, c, m, eng)
                    rope_finish(raw_q, qT, c, m, eng)
                else:
                    rope_finish(proj_mm(xt_k, wk_t, m), kT, c, m, eng)
                    rope_finish(proj_mm(xt_q, wq_t, m), qT, c, m, eng)

            def vproj_stile(xt_v, stl):
                """Project s-tile stl of v into vext [s, (h, e)] layout."""
                ps = projps.tile([128, E], F32, tag="ps")
                for kt in range(8):
                    nc.tensor.matmul(
                        ps[:], lhsT=xt_v[:, kt, (stl % 4) * 128:(stl % 4) * 128 + 128],
                        rhs=wv_t[:, kt, :], start=(kt == 0), stop=(kt == 7),
                    )
                nc.vector.tensor_copy(
                    vext[:, stl, :, 0:64],
                    ps[:].rearrange("p (h e) -> p h e", h=4),
                )

            def wo_unit(stl, n, ob, act_evict=False):
                ps = projps.tile([128, CH], F32, tag="ps")
                for pair in range(2):
                    nc.tensor.matmul(
                        ps[:],
                        lhsT=aT[:, pair, stl * 128:(stl + 1) * 128],
                        rhs=wo_t[:, pair, n * CH:(n + 1) * CH],
                        start=(pair == 0), stop=(pair == 1),
                    )
                if act_evict and n == 1:
                    nc.scalar.copy(ob[:, n, :], ps[:])
                else:
                    nc.vector.tensor_copy(ob[:, n, :], ps[:])
                if n == 1:
                    nc.sync.dma_start(
                        out=out[:].rearrange("(t p) n -> p t n", p=128)[:, stl, :],
                        in_=ob[:].rearrange("p a b -> p (a b)"),
                    )

            def wo_stiles(stls, act_evict=False):
                for stl in stls:
                    ob = obufp.tile([128, 2, CH], BF16, tag="ob")
                    for n in range(2):
                        wo_unit(stl, n, ob, act_evict)

            def attention_pair(c, pair, spair, opsum, prefetch=None,
                               filler_mid=None, filler_post=None,
                               tail=False):
                """Scores+softmax+AV for head-pair `pair` of chunk c.
                AV is emitted one jj-step behind exp (SW pipelining).
                filler_mid runs just before the final AV (absorbs the last
                exp's latency); filler_post runs after normalization."""
                nj = 4 * c + 4
                if prefetch is not None:
                    prefetch()
                o = opsum.tile([128, 2, 4, 128], F32, tag="o")
                pend = []   # [(jj, p_tile dict by pt)]

                def emit_norm(qt):
                    # normalize + transpose qt's s-tile as soon as its
                    # accumulation stopped (frees the o tile earlier and
                    # spreads DVE work away from pair boundaries)
                    stl = 4 * c + qt
                    aq2 = aqp.tile([128, 128], BF16, tag="aq2")
                    for pt in range(2):
                        rrec = normp.tile([128, 1], F32, tag="rrec")
                        nc.vector.reciprocal(rrec[:], o[:, pt, qt, 64:65])
                        nc.vector.tensor_scalar_mul(
                            aq2[:, pt * 64:(pt + 1) * 64], o[:, pt, qt, 0:64], rrec[:])
                    nc.sync.dma_start(
                        out=aT[:, pair, stl * 128:(stl + 1) * 128],
                        in_=aq2[:], transpose=True)
                    if tail:
                        wo_stiles([stl], act_evict=True)

                def emit_av(jj, ptiles):
                    for pt in range(2):
                        head = 2 * pair + pt
                        p = ptiles[pt]
                        for dj in range(2):
                            j = jj + dj
                            for qt in range(4):
                                jmax = 4 * c + qt
                                if j > jmax:
                                    continue
                                nc.tensor.matmul(
                                    o[:, pt, qt, 0:65],
                                    lhsT=p[:, dj, qt * 128:(qt + 1) * 128],
                                    rhs=vext[:, j, head, :],
                                    start=(j == 0 and qt == 0),
                                    stop=(j == jmax),
                                    skip_group_check=True,
                                )
                    if EARLY_NORM or tail:
                        for dj in range(2):
                            qt = jj + dj - 4 * c
                            if 0 <= qt < 4:
                                emit_norm(qt)

                for jj in range(0, nj, 2):
                    ptiles = {}
                    for half, pt in ((0, 0), (64, 1)):
                        sp = spair.tile([128, 2, CH], F32, tag="sp")
                        g1 = 0
                        for dj in range(2):
                            j = jj + dj
                            g = max(0, (j - 4 * c)) * 128
                            if dj == 1:
                                g1 = g
                            nc.tensor.matmul(
                                sp[:, dj, g:CH],
                                lhsT=kT[half:half + 64, pair, j * 128:(j + 1) * 128],
                                rhs=qT[half:half + 64, pair,
                                       c * CH + g:(c + 1) * CH],
                                start=True, stop=True,
                            )
                        g0 = max(0, (jj - 4 * c)) * 128
                        p = pblk.tile([128, 2, CH], BF16, tag="p")
                        spf = sp[:].rearrange("p a b -> p (a b)")
                        pf = p[:].rearrange("p a b -> p (a b)")
                        if g1 == 0:
                            nc.scalar.activation(
                                pf[:, g0:], spf[:, g0:],
                                mybir.ActivationFunctionType.Exp)
                        else:
                            # diagonal pair: skip the unwritten masked span
                            nc.scalar.activation(
                                p[:, 0, g0:], sp[:, 0, g0:],
                                mybir.ActivationFunctionType.Exp)
                            nc.scalar.activation(
                                p[:, 1, g1:], sp[:, 1, g1:],
                                mybir.ActivationFunctionType.Exp)
                        for dj in range(2):
                            j = jj + dj
                            if j >= 4 * c:
                                g = (j - 4 * c) * 128
                                nc.vector.tensor_mul(
                                    p[:, dj, g:g + 128], p[:, dj, g:g + 128], lt_t[:])
                        ptiles[pt] = p
                    if len(pend) >= AV_DEPTH:
                        emit_av(*pend.pop(0))
                    pend.append((jj, ptiles))
                if filler_mid is not None:
                    filler_mid()
                while pend:
                    emit_av(*pend.pop(0))
                if not (EARLY_NORM or tail):
                    for qt in range(4):
                        emit_norm(qt)
                if filler_post is not None:
                    filler_post()

            with (
                tc.tile_pool(name="spair", bufs=2, space="PSUM") as spair,
                tc.tile_pool(name="opsum", bufs=1, space="PSUM") as opsum,
            ):
                # PE warm-up: dummy matmuls on a memset tile (no DMA wait)
                # ramp the PE p-state while the real inputs stream in.
                wps = spair.tile([128, 2, CH], F32, tag="sp")
                for wi in range(30):
                    nc.tensor.matmul(
                        wps[:, 0, 0:128], lhsT=wt[:], rhs=wt[:],
                        start=True, stop=True, skip_group_check=True,
                    )
                ct_r = ct[:].rearrange("(mt p) s -> p mt s", p=128)
                st_r = st[:].rearrange("(mt p) s -> p mt s", p=128)
                pre = {}
                with nc.named_scope("proj_c0"):
                    pre[(0, 'k')] = load_x_chunk(xk, 0, nc.sync)
                    nc.sync.dma_start(out=ct_t[:, :, 0:CH], in_=ct_r[:, :, 0:CH])
                    nc.sync.dma_start(out=st_t[:, :, 0:CH], in_=st_r[:, :, 0:CH])
                    nc.sync.dma_start(out=wq_t[:], in_=wq[:].rearrange("(kt p) e -> p kt e", p=128))
                    pre[(0, 'q')] = load_x_chunk(xq, 0, nc.gpsimd)
                    # chunk-0 m0 rope combine on DVE: it's on the critical
                    # path to the first scores and Pool is ~2x slower
                    proj_rope_pair(pre[(0, 'k')], pre[(0, 'q')], 0, 0,
                                   split=False, eng=nc.vector)
                    if not PROJ_SHIFT:
                        proj_rope_pair(pre[(0, 'k')], pre[(0, 'q')], 0, 1, split=False)
                    nc.sync.dma_start(out=wv_t[:], in_=wv[:].rearrange("(kt p) e -> p kt e", p=128))
                    nc.sync.dma_start(out=lt_t[:], in_=lt[:])
                    nc.gpsimd.dma_start(out=wo_t[:], in_=wo[:].rearrange("(pt p) n -> p pt n", p=128))
                    nc.gpsimd.dma_start(out=ct_t[:, :, CH:S], in_=ct_r[:, :, CH:S])
                    nc.gpsimd.dma_start(out=st_t[:, :, CH:S], in_=st_r[:, :, CH:S])
                    pre[(0, 'v')] = load_x_chunk(xv, 0, nc.sync)
                    for stl in range(4):
                        vproj_stile(pre[(0, 'v')], stl)
                # remaining projection units, emitted one per pair boundary
                # (half a chunk ahead of first use)
                if PROJ_SHIFT:
                    punits = [(0, 1), (1, 0), (1, 1), (2, 0), (2, 1), (3, 0), (3, 1)]
                else:
                    punits = [(1, 0), (1, 1), (2, 0), (2, 1), (3, 0), (3, 1), None]
                punits = [u for u in punits if u is not None]
                for c in range(NCH):
                    with nc.named_scope(f"att_c{c}"):
                        for pair in range(2):
                            def prefetch(c=c, pair=pair):
                                if pair == 0 and c + 1 < NCH:
                                    pre[(c + 1, 'k')] = load_x_chunk(xk, c + 1, nc.sync)
                                    pre[(c + 1, 'q')] = load_x_chunk(xq, c + 1, nc.gpsimd)
                                    pre[(c + 1, 'v')] = load_x_chunk(xv, c + 1, nc.sync)

                            def mid(c=c, pair=pair):
                                # upcoming q/k projection+rope unit; prev
                                # chunk's Wo slice
                                bi = 2 * c + pair
                                if bi < len(punits):
                                    pc, pm = punits[bi]
                                    proj_rope_pair(pre[(pc, 'k')],
                                                   pre[(pc, 'q')], pc, pm, split=False)
                                if c >= 1:
                                    wo_stiles([4 * (c - 1) + 2 * pair,
                                               4 * (c - 1) + 2 * pair + 1])

                            def post(c=c, pair=pair):
                                if c + 1 < NCH:
                                    for stl in (4 * (c + 1) + 2 * pair,
                                                4 * (c + 1) + 2 * pair + 1):
                                        vproj_stile(pre[(c + 1, 'v')], stl)
                            attention_pair(c, pair, spair, opsum,
                                           prefetch, mid, post)
                with nc.named_scope("wo_c3"):
                    wo_stiles(range(4 * (NCH - 1), 4 * NCH))

    nc.compile()
    return nc


def _host_tables():
    inv_freq = 1.0 / (10000.0 ** (np.arange(0, DK, 2, dtype=np.float64) / DK))
    pos = np.arange(S, dtype=np.float64)
    fr = pos[:, None] * inv_freq[None, :]          # [S, 32]
    sc8 = 1.0 / math.sqrt(math.sqrt(DK))           # fold 1/sqrt(DK) as sqrt into q and k
    cosT = (np.cos(fr).T * sc8).astype(np.float32)  # [32, S]
    sinT = (np.sin(fr).T * sc8).astype(np.float32)
    C = np.zeros((E, S), np.float32)
    Sg = np.zeros((E, S), np.float32)
    for hh in range(4):
        C[hh * 64:hh * 64 + 32] = cosT
        C[hh * 64 + 32:hh * 64 + 64] = cosT
        Sg[hh * 64:hh * 64 + 32] = -sinT
        Sg[hh * 64 + 32:hh * 64 + 64] = sinT
    # half-swap permutation for two stacked heads (128 rows)
    R = np.zeros((128, 128), np.float32)
    for hh in range(2):
        for j in range(32):
            R[hh * 64 + j, hh * 64 + 32 + j] = 1.0
            R[hh * 64 + 32 + j, hh * 64 + j] = 1.0
    # within-tile causal staircase: keep k-row p for q-col w iff p <= w
    p = np.arange(128)[:, None]
    w = np.arange(128)[None, :]
    LT = (p <= w).astype(np.float32)
    return C, Sg, R, LT


_program_cache = {}


def kernel(q, k, v, mask, Wq, bq, Wk, bk, Wv, bv, Wo, bo):
    q = np.asarray(q, np.float32)
    k = np.asarray(k, np.float32)
    v = np.asarray(v, np.float32)
    mask = np.asarray(mask)
    Wq, bq = np.asarray(Wq, np.float32), np.asarray(bq, np.float32)
    Wk, bk = np.asarray(Wk, np.float32), np.asarray(bk, np.float32)
    Wv, bv = np.asarray(Wv, np.float32), np.asarray(bv, np.float32)
    Wo, bo = np.asarray(Wo, np.float32), np.asarray(bo, np.float32)

    causal = np.array_equal(
        np.asarray(mask[0, 0], np.int64), np.tril(np.ones((S, S), np.int64)))
    if not causal or np.any(bq) or np.any(bk):
        return _np_reference_fallback(q, k, v, mask, Wq, bq, Wk, bk, Wv, bv, Wo, bo)

    if "nc" not in _program_cache:
        _program_cache["nc"] = _build_program()
    nc = _program_cache["nc"]

    C, Sg, R, LT = _host_tables()
    bf = ml_dtypes.bfloat16

    in_maps = []
    for c in range(8):
        b = c // 4
        h0 = 4 * (c % 4)
        sl = slice(h0 * DK, (h0 + 4) * DK)
        in_maps.append({
            "xq": np.ascontiguousarray(q[b].T).astype(bf),
            "xk": np.ascontiguousarray(k[b].T).astype(bf),
            "xv": np.ascontiguousarray(v[b].T).astype(bf),
            "wq": np.ascontiguousarray(Wq[sl].T).astype(bf),
            "wk": np.ascontiguousarray(Wk[sl].T).astype(bf),
            "wv": np.ascontiguousarray(Wv[sl].T).astype(bf),
            "wo": np.ascontiguousarray(Wo[:, sl].T).astype(bf),
            "ct": C.astype(bf),
            "st": Sg.astype(bf),
            "rt": R.astype(bf),
            "lt": LT.astype(bf),
        })

    res = run_bass_kernel_spmd(nc, in_maps, core_ids=list(range(8)))

    out = np.zeros((B, S, D), np.float32)
    for c in range(8):
        out[c // 4] += res.results[c]["out"].astype(np.float32)
    # bv folds through softmax (rows sum to 1) and Wo; bo direct.
    out += (bv @ Wo.T + bo)[None, None, :]
    return out
